# revision 2
# baseline (speedup 1.0000x reference)
"""Trainium2 Bass kernel for the windowed-local-attention block.

Contract: kernel(**inputs) takes the FULL unsharded inputs
(x: (8,8192,512) f32, w_q/w_k/w_v: (512,512) f32, b_q/b_k/b_v: (512,) f32)
and returns the full output (8,8192,512) f32.

Strategy: data-parallel over batch. B == n_cores == 8, and the attention
is strictly within a batch element, so each core independently processes
one (8192, 512) sequence; no collectives are needed.

On-chip schedule per core (bf16 matmuls, fp32 accumulation):
  for each supertile of 512 tokens (4 windows of 128):
    load x^T (feature-major, transposed host-side)
    q^T, k^T   = W^T-slices . x^T       (feature-major, for the S matmul)
    v          = x^T-slices . Wv^T      (token-major, for the PV matmul)
    for each window:
      S   = q^T_w^T . k^T_[w-1,w]       (128 x 256, fp32 PSUM)
      S  += causal/pad mask             (DVE, in-place in PSUM)
      P   = exp(S), l = rowsum(P)       (ScalarE with fused accumulate)
      P^T = PE transpose
      O   = P^T^T . v_[w-1,w]           (fp32 PSUM)
      out = O * (1/l)                   (per-row scale during PSUM->SBUF copy)
"""

import os
import sys

import numpy as np

for _p in ("/opt/trn_rl_repo",):
    if _p not in sys.path and os.path.isdir(_p):
        sys.path.insert(0, _p)

import ml_dtypes

import concourse.bass as bass
import concourse.mybir as mybir
import concourse.tile as tile
from concourse import bacc
from concourse.bass import ds, ts
from concourse.bass_utils import run_bass_kernel_spmd
from concourse.masks import make_identity

FP32 = mybir.dt.float32
BF16 = mybir.dt.bfloat16

D = 512          # model dim
WS = 128         # attention window size
ST = 512         # tokens per supertile (4 windows)
NCORES = 8
NEG = -1.0e9     # mask fill; exp(NEG + O(1)) == 0.0 in fp32


def build_nc(ntok: int):
    """Build + compile the per-core Bass program for `ntok` tokens."""
    nst = ntok // ST
    nc = bacc.Bacc(
        "TRN2", target_bir_lowering=False, debug=False, num_devices=NCORES
    )

    xT_d = nc.dram_tensor("xT", [D, ntok], BF16, kind="ExternalInput").ap()
    wq_d = nc.dram_tensor("wqT", [D, D], BF16, kind="ExternalInput").ap()
    wk_d = nc.dram_tensor("wkT", [D, D], BF16, kind="ExternalInput").ap()
    wv_d = nc.dram_tensor("wvT", [D, D], BF16, kind="ExternalInput").ap()
    bq_d = nc.dram_tensor("bq", [D], FP32, kind="ExternalInput").ap()
    bk_d = nc.dram_tensor("bk", [D], FP32, kind="ExternalInput").ap()
    bv_d = nc.dram_tensor("bv", [D], FP32, kind="ExternalInput").ap()
    # mask[0]: windows w>0 (prev window fully visible); mask[1]: w==0 (prev
    # window is padding -> fully masked). Cols 128:256 carry the causal
    # upper-triangle in both.
    mask_d = nc.dram_tensor("mask", [2, WS, 2 * WS], FP32, kind="ExternalInput").ap()
    out_d = nc.dram_tensor("out", [ntok, D], FP32, kind="ExternalOutput").ap()

    with tile.TileContext(nc) as tc:
        with (
            tc.tile_pool(name="const", bufs=1) as cpool,
            tc.tile_pool(name="sb", bufs=2) as sb,
            tc.tile_pool(name="ps", bufs=2, space="PSUM") as ps,
        ):
            # ---- constants (loaded once) ----
            wq_sb = cpool.tile([128, 4, D], BF16, name="wq_sb")
            nc.sync.dma_start(
                out=wq_sb, in_=wq_d.rearrange("(c p) d -> p c d", p=128)
            )
            wk_sb = cpool.tile([128, 4, D], BF16, name="wk_sb")
            nc.sync.dma_start(
                out=wk_sb, in_=wk_d.rearrange("(c p) d -> p c d", p=128)
            )
            wv_sb = cpool.tile([128, 4, D], BF16, name="wv_sb")
            nc.sync.dma_start(
                out=wv_sb, in_=wv_d.rearrange("(c p) d -> p c d", p=128)
            )
            bq_sb = cpool.tile([128, 4], FP32, name="bq_sb")
            nc.sync.dma_start(out=bq_sb, in_=bq_d.rearrange("(m p) -> p m", p=128))
            bk_sb = cpool.tile([128, 4], FP32, name="bk_sb")
            nc.sync.dma_start(out=bk_sb, in_=bk_d.rearrange("(m p) -> p m", p=128))
            # b_v broadcast across partitions: (128, 512) with partition stride 0
            bvb = cpool.tile([128, D], FP32, name="bvb")
            nc.sync.dma_start(
                out=bvb,
                in_=bass.AP(
                    tensor=bv_d.tensor, offset=bv_d.offset, ap=[[0, 128]] + bv_d.ap
                ),
            )
            masks = cpool.tile([128, 2, 2 * WS], FP32, name="masks")
            nc.sync.dma_start(
                out=masks, in_=mask_d.rearrange("k p j -> p k j")
            )
            ident = cpool.tile([128, 128], BF16, name="ident")
            make_identity(nc, ident)

            prev_kt = None
            prev_v = None
            for t in range(nst):
                # x^T for this supertile: 4 feature chunks x (128, 512 tokens)
                xt = sb.tile([128, 4, ST], BF16, tag="xt", bufs=6, name=f"xt{t}")
                nc.sync.dma_start(
                    out=xt,
                    in_=xT_d[:, ds(t * ST, ST)].rearrange("(c p) s -> p c s", p=128),
                )

                qt = sb.tile([128, 4, ST], BF16, tag="qt", bufs=2, name=f"qt{t}")
                # k^T with a one-window halo on the left: cols [0:128) hold the
                # last window of the previous supertile.
                kt = sb.tile([128, 4, ST + WS], BF16, tag="kt", bufs=2, name=f"kt{t}")

                for m in range(4):
                    pq = ps.tile([128, ST], FP32, tag="proj", name=f"pq{t}_{m}")
                    for c in range(4):
                        nc.tensor.matmul(
                            pq,
                            lhsT=wq_sb[:, c, ds(m * 128, 128)],
                            rhs=xt[:, c, :],
                            start=(c == 0),
                            stop=(c == 3),
                        )
                    nc.scalar.activation(
                        out=qt[:, m, :],
                        in_=pq,
                        func=mybir.ActivationFunctionType.Identity,
                        bias=bq_sb[:, m : m + 1],
                        scale=1.0,
                    )
                for m in range(4):
                    pk = ps.tile([128, ST], FP32, tag="proj", name=f"pk{t}_{m}")
                    for c in range(4):
                        nc.tensor.matmul(
                            pk,
                            lhsT=wk_sb[:, c, ds(m * 128, 128)],
                            rhs=xt[:, c, :],
                            start=(c == 0),
                            stop=(c == 3),
                        )
                    nc.scalar.activation(
                        out=kt[:, m, ds(WS, ST)],
                        in_=pk,
                        func=mybir.ActivationFunctionType.Identity,
                        bias=bk_sb[:, m : m + 1],
                        scale=1.0,
                    )
                if t == 0:
                    # left halo of the first supertile is padding; zero it so
                    # the (masked-anyway) S contributions stay finite
                    nc.vector.memset(kt[:, :, ds(0, WS)], 0.0)
                else:
                    for m in range(4):
                        nc.vector.tensor_copy(
                            out=kt[:, m, ds(0, WS)], in_=prev_kt[:, m, ds(ST, WS)]
                        )
                if t == 0:
                    v_pad = sb.tile([128, D], BF16, tag="v", bufs=9, name="v_pad")
                    nc.vector.memset(v_pad, 0.0)
                    prev_v = v_pad

                vs = []
                for j in range(4):
                    pv = ps.tile([128, D], FP32, tag="proj", name=f"pv{t}_{j}")
                    for c in range(4):
                        nc.tensor.matmul(
                            pv,
                            lhsT=xt[:, c, ds(j * 128, 128)],
                            rhs=wv_sb[:, c, :],
                            start=(c == 0),
                            stop=(c == 3),
                        )
                    vj = sb.tile([128, D], BF16, tag="v", bufs=9, name=f"v{t}_{j}")
                    nc.vector.tensor_add(out=vj, in0=pv, in1=bvb)
                    vs.append(vj)

                for j in range(4):
                    w = t * 4 + j
                    vprev = vs[j - 1] if j > 0 else prev_v
                    s = ps.tile([128, 2 * WS], FP32, tag="s", name=f"s{w}")
                    for m in range(4):
                        nc.tensor.matmul(
                            s,
                            lhsT=qt[:, m, ds(j * WS, WS)],
                            rhs=kt[:, m, ds(j * WS, 2 * WS)],
                            start=(m == 0),
                            stop=(m == 3),
                        )
                    nc.vector.tensor_add(
                        out=s, in0=s, in1=masks[:, 1 if w == 0 else 0, :]
                    )
                    p = sb.tile([128, 2 * WS], BF16, tag="p", bufs=3, name=f"p{w}")
                    l = sb.tile([128, 1], FP32, tag="l", bufs=4, name=f"l{w}")
                    nc.scalar.activation(
                        out=p,
                        in_=s,
                        func=mybir.ActivationFunctionType.Exp,
                        bias=0.0,
                        scale=1.0,
                        accum_out=l,
                    )
                    r = sb.tile([128, 1], FP32, tag="r", bufs=4, name=f"r{w}")
                    nc.vector.reciprocal(r, l)
                    ptp = ps.tile([128, 2 * WS], BF16, tag="pt", name=f"ptp{w}")
                    nc.tensor.transpose(ptp[:, ds(0, WS)], p[:, ds(0, WS)], ident)
                    nc.tensor.transpose(ptp[:, ds(WS, WS)], p[:, ds(WS, WS)], ident)
                    ptsb = sb.tile(
                        [128, 2 * WS], BF16, tag="ptsb", bufs=3, name=f"ptsb{w}"
                    )
                    nc.scalar.copy(ptsb, ptp)
                    o = ps.tile([128, D], FP32, tag="o", name=f"o{w}")
                    nc.tensor.matmul(
                        o, lhsT=ptsb[:, ds(0, WS)], rhs=vprev, start=True, stop=False
                    )
                    nc.tensor.matmul(
                        o, lhsT=ptsb[:, ds(WS, WS)], rhs=vs[j], start=False, stop=True
                    )
                    osb = sb.tile([128, D], FP32, tag="osb", bufs=4, name=f"osb{w}")
                    nc.vector.tensor_scalar_mul(osb, o, r)
                    nc.sync.dma_start(out=out_d[ds(w * WS, WS), :], in_=osb)

                prev_kt = kt
                prev_v = vs[3]

    nc.compile()
    return nc


_NC_CACHE: dict[int, object] = {}


def _get_nc(ntok: int):
    if ntok not in _NC_CACHE:
        _NC_CACHE[ntok] = build_nc(ntok)
    return _NC_CACHE[ntok]


def _host_prep(x, w_q, b_q, w_k, b_k, w_v, b_v):
    """Build the per-core input maps (host-side shard + preprocess)."""
    bf = ml_dtypes.bfloat16
    b, ntok, d = x.shape
    assert d == D
    scale = float(d) ** -0.5

    wq = np.ascontiguousarray(w_q.T * scale).astype(bf)
    wk = np.ascontiguousarray(w_k.T).astype(bf)
    wv = np.ascontiguousarray(w_v.T).astype(bf)
    bq = (b_q * scale).astype(np.float32)
    bk = b_k.astype(np.float32)
    bv = b_v.astype(np.float32)

    causal = np.triu(np.full((WS, WS), NEG, np.float32), 1)
    mask = np.zeros((2, WS, 2 * WS), np.float32)
    mask[:, :, WS:] = causal
    mask[1, :, :WS] = NEG

    in_maps = []
    for i in range(b):
        in_maps.append(
            {
                "xT": np.ascontiguousarray(x[i].T).astype(bf),
                "wqT": wq,
                "wkT": wk,
                "wvT": wv,
                "bq": bq,
                "bk": bk,
                "bv": bv,
                "mask": mask,
            }
        )
    return in_maps


def run_on_hw(x, w_q, b_q, w_k, b_k, w_v, b_v, trace=False):
    """Run on the 8 NeuronCores; returns (output, BassKernelResults)."""
    b, ntok, _ = x.shape
    assert b == NCORES, f"batch {b} != {NCORES} cores"
    nc = _get_nc(ntok)
    in_maps = _host_prep(x, w_q, b_q, w_k, b_k, w_v, b_v)
    res = run_bass_kernel_spmd(nc, in_maps, list(range(NCORES)), trace=trace)
    out = np.stack([res.results[i]["out"] for i in range(NCORES)]).astype(np.float32)
    return out, res


def kernel(x, w_q, b_q, w_k, b_k, w_v, b_v):
    out, _ = run_on_hw(
        np.asarray(x, np.float32),
        np.asarray(w_q, np.float32),
        np.asarray(b_q, np.float32),
        np.asarray(w_k, np.float32),
        np.asarray(b_k, np.float32),
        np.asarray(w_v, np.float32),
        np.asarray(b_v, np.float32),
    )
    return out
